# revision 18
# baseline (speedup 1.0000x reference)
"""Trainium2 Bass kernel for a 3-layer char-LSTM (B=128, T=512, H=512, V=96).

Sharding: data-parallel over the batch dim across 8 NeuronCores (16 rows each).
Per core everything is SBUF-resident. Per timestep each layer runs
batch-stationary matmuls in 128x32 column-tiled mode: the 16-column x^T
stationary is replicated into 4 PE column groups, each streaming a different
512-column slice of the (gate-permuted) weight matrix into its own PSUM
partition quadrant. Gates are ordered [i, f, o, j] so one sigmoid activation
(with a per-partition forget bias) covers i/f/o and one tanh covers j.
The layer-0 one-hot contribution (plus b0 via an appended ones-row) is just
another accumulation burst with a [97, 16] stationary. h is staged into four
partition groups and re-transposed with a single [128,128] PE transpose per
layer to become the next stationary. Logits are computed per step on column
group 0; softmax/probs/argmax run in a post-loop pass over [128, 96] tiles.
"""

import numpy as np

FORGET_BIAS = 1.0
V = 96
H = 512
G = 4 * H  # 2048
B_FULL = 128
T_FULL = 512
N_CORES = 8
B = B_FULL // N_CORES  # 16 per core
F32 = None  # set after import


def _gate_perm():
    # reference gate order: i, j, f, o (each H wide) -> ours: i, f, o, j
    idx = np.arange(G).reshape(4, H)
    return np.concatenate([idx[0], idx[2], idx[3], idx[1]])


def _build(T, nk_l0=4, with_b1=False, with_b2=False, with_bout=False):
    from concourse import bacc, mybir, tile
    from concourse.masks import make_identity
    import concourse.bass as bass

    dt = mybir.dt
    AF = mybir.ActivationFunctionType
    ALU = mybir.AluOpType

    nc = bacc.Bacc("TRN2", target_bir_lowering=False, debug=False,
                   num_devices=N_CORES)

    NG = T // 8  # logit groups of 8 steps

    # ---------------- DRAM I/O ----------------
    d_w0x = nc.declare_dram_parameter("w0x", [97, G], dt.float32, isOutput=False)
    d_wh0 = nc.declare_dram_parameter("wh0", [512, G], dt.float32, isOutput=False)
    d_w1 = nc.declare_dram_parameter("w1", [1024, G], dt.float32, isOutput=False)
    d_w2 = nc.declare_dram_parameter("w2", [1024, G], dt.float32, isOutput=False)
    d_wout = nc.declare_dram_parameter("wout", [512, V], dt.float32, isOutput=False)
    d_oht = nc.declare_dram_parameter("oht", [97, T * B], dt.float32, isOutput=False)
    d_lg = nc.declare_dram_parameter("lg", [NG, B, 8 * V], dt.float32, isOutput=True)
    d_pr = nc.declare_dram_parameter("pr", [NG, B, 8 * V], dt.float32, isOutput=True)
    d_cm = nc.declare_dram_parameter("cm", [NG, B * 8], dt.int32, isOutput=True)

    extra = {}
    if with_b1:
        extra["b1"] = nc.declare_dram_parameter("b1", [1, G], dt.float32, isOutput=False)
    if with_b2:
        extra["b2"] = nc.declare_dram_parameter("b2", [1, G], dt.float32, isOutput=False)
    if with_bout:
        extra["bout"] = nc.declare_dram_parameter("bout", [1, V], dt.float32, isOutput=False)

    from contextlib import ExitStack

    with tile.TileContext(nc) as tc, ExitStack() as ctx:
        const = ctx.enter_context(tc.tile_pool(name="const", bufs=1))
        state = ctx.enter_context(tc.tile_pool(name="state", bufs=1))
        ws = ctx.enter_context(tc.tile_pool(name="ws", bufs=2))
        ohwin_pool = ctx.enter_context(tc.tile_pool(name="ohwin", bufs=2))
        ring_pool = ctx.enter_context(tc.tile_pool(name="ring", bufs=2))
        zpsum = ctx.enter_context(tc.tile_pool(name="zpsum", bufs=3, space="PSUM"))
        tpsum = ctx.enter_context(tc.tile_pool(name="tpsum", bufs=2, space="PSUM"))
        lpsum = ctx.enter_context(tc.tile_pool(name="lpsum", bufs=2, space="PSUM"))
        post = ctx.enter_context(tc.tile_pool(name="post", bufs=3))

        # ---------------- resident weights ----------------
        w0x_sb = const.tile([97, G], dt.float32)
        nc.sync.dma_start(w0x_sb[:], d_w0x[:])
        wh0_sb = const.tile([128, 4 * G], dt.float32)
        for k in range(4):
            nc.sync.dma_start(wh0_sb[:, k * G:(k + 1) * G],
                              d_wh0[k * 128:(k + 1) * 128, :])
        w1_sb = const.tile([128, 8 * G], dt.float32)
        w2_sb = const.tile([128, 8 * G], dt.float32)
        for k in range(8):
            nc.sync.dma_start(w1_sb[:, k * G:(k + 1) * G],
                              d_w1[k * 128:(k + 1) * 128, :])
            nc.sync.dma_start(w2_sb[:, k * G:(k + 1) * G],
                              d_w2[k * 128:(k + 1) * 128, :])
        wout_sb = const.tile([128, 4 * V], dt.float32)
        for k in range(4):
            nc.sync.dma_start(wout_sb[:, k * V:(k + 1) * V],
                              d_wout[k * 128:(k + 1) * 128, :])
        WIN = min(64, T)  # one-hot stationary window (steps)

        bias_sb = {}
        ones1 = None
        if extra:
            ones1 = const.tile([1, B], dt.float32)
            nc.vector.memset(ones1[:], 1.0)
            for name, dram in extra.items():
                t_ = const.tile([1, dram.shape[1]], dt.float32)
                nc.sync.dma_start(t_[:], dram[:])
                bias_sb[name] = t_

        identity = const.tile([128, 128], dt.float32)
        make_identity(nc, identity[:])

        fbias = const.tile([128, 1], dt.float32)
        nc.vector.memset(fbias[:], 0.0)
        nc.vector.memset(fbias[32:48, :], FORGET_BIAS)

        # ---------------- recurrent state ----------------
        cS, hTS, hstg = [], [], []
        for l in range(3):
            c_t = state.tile([16, H], dt.float32, tag=f"c{l}")
            nc.vector.memset(c_t[:], 0.0)
            cS.append(c_t)
            h_t = state.tile([128, 4, 16], dt.float32, tag=f"hT{l}")
            nc.vector.memset(h_t[:], 0.0)
            hTS.append(h_t)
            s_t = state.tile([128, 128], dt.float32, tag=f"hstg{l}")
            nc.vector.memset(s_t[:], 0.0)
            hstg.append(s_t)

        w_sb = [wh0_sb, w1_sb, w2_sb]
        nks = [4, 8, 8]

        ring = None
        ohw = None
        for t in range(T):
            if t % WIN == 0:
                ohw = ohwin_pool.tile([97, WIN * B], dt.float32, tag="ohw")
                nc.sync.dma_start(ohw[:], d_oht[:, t * B:(t + WIN) * B])
            for l in range(3):
                zps = zpsum.tile([128, 512], dt.float32, tag="z")
                # ---- matmul bursts: 4 column-tiles, K-chunk accumulation ----
                # h-recurrent K-chunks first (start=True) so they only wait on
                # this layer's own transpose, not on the previous layer.
                nk = nks[l]
                if l == 0:
                    korder = list(range(nk)) + ["oh"]
                else:
                    korder = list(range(4, nk)) + list(range(4))
                bias_t = bias_sb.get(f"b{l}") if l > 0 else None
                for pos, k in enumerate(korder):
                    first = pos == 0
                    last = (pos == len(korder) - 1) and bias_t is None
                    if k == "oh":
                        lhsT = ohw[:, (t % WIN) * B:((t % WIN) + 1) * B]
                        wslice = lambda c: w0x_sb[:, c * 512:(c + 1) * 512]
                    else:
                        if l == 0:
                            lhsT = hTS[0][:, k, :]
                        elif k < 4:
                            lhsT = hTS[l - 1][:, k, :]
                        else:
                            lhsT = hTS[l][:, k - 4, :]
                        wslice = (lambda kk: lambda c: w_sb[l][
                            :, kk * G + c * 512:kk * G + (c + 1) * 512])(k)
                    for c in range(4):
                        nc.tensor.matmul(
                            zps[32 * c:32 * c + 16, :], lhsT, wslice(c),
                            start=first, stop=last,
                            tile_position=(0, 32 * c))
                if bias_t is not None:
                    bt = bias_t
                    for c in range(4):
                        nc.tensor.matmul(
                            zps[32 * c:32 * c + 16, :], ones1[:],
                            bt[:, c * 512:(c + 1) * 512],
                            start=False, stop=True, tile_position=(0, 32 * c))

                # ---- gate math ----
                # zps partition groups: [0:16]=i, [32:48]=f, [64:80]=o, [96:112]=j
                # sigmoid i/f/o in place in PSUM (f gets +FORGET_BIAS via fbias)
                nc.scalar.activation(zps[0:80, :], zps[0:80, :], AF.Sigmoid,
                                     bias=fbias[0:80, :])
                ws2 = ws.tile([48, 1024], dt.float32, tag="ws2")
                tct, tj = ws2[0:16, 0:512], ws2[0:16, 512:1024]
                tmp, c2 = ws2[32:48, 0:512], ws2[32:48, 512:1024]
                nc.scalar.activation(tj, zps[96:112, :], AF.Tanh)
                nc.vector.tensor_tensor(tmp, tj, zps[0:16, :], ALU.mult)
                nc.vector.tensor_tensor(c2, cS[l][:], zps[32:48, :], ALU.mult)
                nc.gpsimd.tensor_tensor(cS[l][:], tmp, c2, ALU.add)
                nc.scalar.activation(tct, cS[l][:], AF.Tanh)
                # ---- stage h into 4 partition groups, transpose back ----
                for g in range(4):
                    nc.vector.tensor_tensor(
                        hstg[l][32 * g:32 * g + 16, :],
                        tct[:, 128 * g:128 * (g + 1)],
                        zps[64:80, 128 * g:128 * (g + 1)], ALU.mult)
                ptr = tpsum.tile([128, 128], dt.float32, tag="ptr")
                nc.tensor.transpose(ptr[:], hstg[l][:], identity[:])
                nc.scalar.copy(
                    hTS[l][:],
                    ptr[:].rearrange("p (g q) -> p g q", g=4)[:, :, 0:16])

            # ---- logits for this step (column group 0 only) ----
            lps = lpsum.tile([16, V], dt.float32, tag="lg")
            for k in range(4):
                nc.tensor.matmul(lps[:], hTS[2][:, k, :],
                                 wout_sb[:, k * V:(k + 1) * V],
                                 start=(k == 0),
                                 stop=(k == 3 and "bout" not in bias_sb),
                                 tile_position=(0, 0))
            if "bout" in bias_sb:
                nc.tensor.matmul(lps[:], ones1[:], bias_sb["bout"][:],
                                 start=False, stop=True, tile_position=(0, 0))
            if t % 8 == 0:
                ring = ring_pool.tile([16, 8 * V], dt.float32, tag="ring")
            nc.scalar.copy(ring[:, (t % 8) * V:(t % 8 + 1) * V], lps[:])
            if t % 8 == 7:
                nc.sync.dma_start(d_lg[t // 8], ring[:])

        # ---------------- post pass: softmax / probs / argmax ----------------
        for g in range(NG):
            ld = post.tile([128, V], dt.float32, tag="ld")
            nc.sync.dma_start(ld[:], d_lg[g].rearrange("b (s v) -> (b s) v", v=V))
            mx = post.tile([128, 8], dt.float32, tag="mx")
            midx = post.tile([128, 8], dt.uint32, tag="midx")
            nc.vector.max_with_indices(mx[:], midx[:], ld[:])
            nm = post.tile([128, 1], dt.float32, tag="nm")
            nc.vector.tensor_scalar_mul(nm[:], mx[:, 0:1], -1.0)
            ex = post.tile([128, V], dt.float32, tag="ex")
            sm = post.tile([128, 1], dt.float32, tag="sm")
            nc.scalar.activation(ex[:], ld[:], AF.Exp, bias=nm[:],
                                 accum_out=sm[:])
            rc = post.tile([128, 1], dt.float32, tag="rc")
            nc.vector.reciprocal(rc[:], sm[:])
            pb = post.tile([128, V], dt.float32, tag="pb")
            nc.vector.tensor_scalar_mul(pb[:], ex[:], rc[:])
            nc.sync.dma_start(d_pr[g].rearrange("b (s v) -> (b s) v", v=V), pb[:])
            nc.sync.dma_start(d_cm[g], midx[:, 0:1].bitcast(dt.int32))

    nc.compile()
    return nc


_NC_CACHE = {}
TRACE = False          # set True to profile; results land in LAST_RESULT
LAST_RESULT = None


def _patch_pjrt_transfer():
    """Replace bass2jax.run_bass_via_pjrt's numpy-arg path with explicit
    sharded jax.device_put: the jit argument-upload path over axon moves
    incompressible data at ~0.5 MB/s, while device_put runs at ~40 MB/s.
    Logic otherwise mirrors the original multi-core branch."""
    from concourse import bass2jax
    from concourse import mybir
    import jax
    from jax.sharding import Mesh, PartitionSpec, NamedSharding
    from jax.experimental.shard_map import shard_map

    if getattr(bass2jax, "_fast_put_patched", False):
        return
    orig = bass2jax.run_bass_via_pjrt

    def fast(nc, in_maps, n_cores):
        if n_cores == 1:
            return orig(nc, in_maps, n_cores)
        bass2jax.install_neuronx_cc_hook()
        partition_name = (nc.partition_id_tensor.name
                          if nc.partition_id_tensor else None)
        in_names, out_names, out_avals, zero_outs = [], [], [], []
        for alloc in nc.m.functions[0].allocations:
            if not isinstance(alloc, mybir.MemoryLocationSet):
                continue
            name = alloc.memorylocations[0].name
            if alloc.kind == "ExternalInput":
                if name != partition_name:
                    in_names.append(name)
            elif alloc.kind == "ExternalOutput":
                shape = tuple(alloc.tensor_shape)
                dtype = mybir.dt.np(alloc.dtype)
                out_names.append(name)
                out_avals.append(jax.core.ShapedArray(shape, dtype))
                zero_outs.append(np.zeros(shape, dtype))
        n_params = len(in_names)
        n_outs = len(out_avals)
        in_names.extend(out_names)
        if partition_name is not None:
            in_names.append(partition_name)

        def _body(*args):
            operands = list(args)
            if partition_name is not None:
                operands.append(bass2jax.partition_id_tensor())
            return tuple(bass2jax._bass_exec_p.bind(
                *operands, out_avals=tuple(out_avals),
                in_names=tuple(in_names), out_names=tuple(out_names),
                lowering_input_output_aliases=(),
                sim_require_finite=True, sim_require_nnan=True, nc=nc))

        devices = jax.devices()[:n_cores]
        mesh = Mesh(np.asarray(devices), ("core",))
        sh = NamedSharding(mesh, PartitionSpec("core"))
        donate = tuple(range(n_params, n_params + n_outs))
        sharded = jax.jit(
            shard_map(_body, mesh=mesh,
                      in_specs=(PartitionSpec("core"),) * (n_params + n_outs),
                      out_specs=(PartitionSpec("core"),) * n_outs,
                      check_rep=False),
            donate_argnums=donate, keep_unused=True)
        concat_in = [
            jax.device_put(
                np.concatenate([np.asarray(m[in_names[i]]) for m in in_maps],
                               axis=0), sh)
            for i in range(n_params)]
        concat_zeros = [
            jax.device_put(
                np.zeros((n_cores * z.shape[0], *z.shape[1:]), z.dtype), sh)
            for z in zero_outs]
        out_arrs = sharded(*concat_in, *concat_zeros)
        return [
            {name: np.asarray(out_arrs[i]).reshape(
                n_cores, *out_avals[i].shape)[c]
             for i, name in enumerate(out_names)}
            for c in range(n_cores)]

    bass2jax.run_bass_via_pjrt = fast
    bass2jax._fast_put_patched = True


def _get_nc(T, key):
    if (T, key) not in _NC_CACHE:
        _NC_CACHE[(T, key)] = _build(T, with_b1=key[0], with_b2=key[1],
                                     with_bout=key[2])
    return _NC_CACHE[(T, key)]


def kernel(char_in, W0, b0, W1, b1, W2, b2, Wout, bout):
    from concourse.bass_utils import run_bass_kernel_spmd
    _patch_pjrt_transfer()

    char_in = np.asarray(char_in)
    B_full, T = char_in.shape
    perm = _gate_perm()

    w0x = np.empty((97, G), np.float32)
    w0x[:96] = np.asarray(W0, np.float32)[:96, perm]
    w0x[96] = np.asarray(b0, np.float32)[perm]
    wh0 = np.ascontiguousarray(np.asarray(W0, np.float32)[96:, perm])
    w1 = np.ascontiguousarray(np.asarray(W1, np.float32)[:, perm])
    w2 = np.ascontiguousarray(np.asarray(W2, np.float32)[:, perm])
    wout = np.ascontiguousarray(np.asarray(Wout, np.float32))

    with_b1 = bool(np.any(np.asarray(b1)))
    with_b2 = bool(np.any(np.asarray(b2)))
    with_bout = bool(np.any(np.asarray(bout)))
    nc = _get_nc(T, (with_b1, with_b2, with_bout))

    in_maps = []
    for core in range(N_CORES):
        bs = slice(core * B, (core + 1) * B)
        ch = char_in[bs]  # [16, T]
        oht = np.zeros((97, T * B), np.float32)
        tt, bb = np.meshgrid(np.arange(T), np.arange(B), indexing="ij")
        oht[ch.T.ravel(), (tt * B + bb).ravel()] = 1.0
        oht[96, :] = 1.0
        m = {"w0x": w0x, "wh0": wh0, "w1": w1, "w2": w2, "wout": wout,
             "oht": oht}
        if with_b1:
            m["b1"] = np.asarray(b1, np.float32)[perm][None, :]
        if with_b2:
            m["b2"] = np.asarray(b2, np.float32)[perm][None, :]
        if with_bout:
            m["bout"] = np.asarray(bout, np.float32)[None, :]
        in_maps.append(m)

    global LAST_RESULT
    kw = {"trace": True} if TRACE else {}
    LAST_RESULT = run_bass_kernel_spmd(nc, in_maps, list(range(N_CORES)), **kw)
    res = LAST_RESULT.results

    NG = T // 8
    logits = np.empty((B_full, T, V), np.float32)
    probs = np.empty((B_full, T, V), np.float32)
    char_max = np.empty((B_full, T), np.int32)
    for core in range(N_CORES):
        bs = slice(core * B, (core + 1) * B)
        lg = res[core]["lg"].reshape(NG, B, 8, V).transpose(1, 0, 2, 3)
        logits[bs] = lg.reshape(B, T, V)
        pr = res[core]["pr"].reshape(NG, B, 8, V).transpose(1, 0, 2, 3)
        probs[bs] = pr.reshape(B, T, V)
        cm = res[core]["cm"].reshape(NG, B, 8).transpose(1, 0, 2)
        char_max[bs] = cm.reshape(B, T)
    return logits, probs, char_max
